# revision 1
# baseline (speedup 1.0000x reference)
"""GNN message-passing ConvNet layer on 8 TRN2 NeuronCores (Bass/Tile).

Computes, for x [B=4, N=4096, D=128], adj_mat [B, N, N] (0/1 floats),
U [D, D]:
    mask = (adj_mat > 0)
    deg[b, i] = sum_j adj_mat[b, j, i]
    agg[b, i, :] = sum_j mask[b, j, i] * x[b, j, :]
    out = relu((agg @ U) / deg[..., None])

Sharding (chosen over the all-reduce hint): split the *destination* node
axis i instead of the source axis j. Core c handles batch c//2 and
destination half c%2: it reads its own column slice adj[b, :, i0:i0+2048]
(32 MiB) plus all of x[b] (2 MiB) and computes its output slice with no
collectives. Traffic per core is the same as j-sharding but there is no
all-reduce, no partial-sum exchange, and per-core work is identical.

Per-core kernel (memory-bound, ~33 MiB HBM reads):
  - adj streams HBM -> SBUF in 4 MiB DMAs and through the PE as the moving
    operand in float32r (1 cycle/row at free-dim 512; adj is 0/1 so the
    fp32r rounding only touches x).
  - X 128x128 tiles are the stationary operand; aggT[d, i] accumulates in
    PSUM over the 32 j-tiles. A parallel ones[128,1]-stationary accumulation
    produces deg[1, i] in a second PSUM bank (exact: 0/1 sums).
  - i is processed in 4 rounds of 512 so PSUM (8 banks) holds agg+deg+out2
    double-buffered and each round's tail overlaps the next round's DMA.
  - Tail per round: recip(deg) -> partition-broadcast -> scale aggT on the
    free axis -> 4 U-matmuls (plain fp32) -> fused ReLU on ScalarE -> one
    256 KiB store.
"""

import os
import sys

for _p in ("/opt/trn_rl_repo",):
    if _p not in sys.path and os.path.isdir(_p):
        sys.path.insert(0, _p)

from contextlib import ExitStack

import numpy as np

B, N, D = 4, 4096, 128
P = 128
N_CORES = 8

_PROG = None


def _build_program(n=N, i_core=N // 2, d=D, w=512, jt_per_dma=8):
    from concourse import mybir, tile, bacc

    f32 = mybir.dt.float32
    f32r = mybir.dt.float32r
    n_jt = n // P
    n_rounds = i_core // w
    n_dma = n_jt // jt_per_dma
    n_ot = w // P

    nc = bacc.Bacc(
        "TRN2",
        target_bir_lowering=False,
        debug=False,
        enable_asserts=True,
        num_devices=N_CORES,
    )
    adj_d = nc.dram_tensor("adj_s", [n, i_core], f32r, kind="ExternalInput")
    # x pre-packed on host to partition-major [128, n_jt, d] so the load is
    # one contiguous DMA (2 KiB+ descriptors) instead of 512 B row gathers.
    x_d = nc.dram_tensor("x_sp", [P, n_jt, d], f32r, kind="ExternalInput")
    u_d = nc.dram_tensor("U", [d, d], f32, kind="ExternalInput")
    ones_d = nc.dram_tensor("ones_c", [P, 1], f32r, kind="ExternalInput")
    # output in partition-major [128, i_core//P, d]; host unpacks.
    out_d = nc.dram_tensor("out_sp", [P, i_core // P, d], f32, kind="ExternalOutput")

    with tile.TileContext(nc, trace_sim=False) as tc, ExitStack() as ctx:
        const_pool = ctx.enter_context(tc.tile_pool(name="const", bufs=1))
        adj_pool = ctx.enter_context(tc.tile_pool(name="adj", bufs=6))
        scale_pool = ctx.enter_context(tc.tile_pool(name="scale", bufs=2))
        out_pool = ctx.enter_context(tc.tile_pool(name="out", bufs=2))
        small_pool = ctx.enter_context(tc.tile_pool(name="small", bufs=2))
        ps_agg = ctx.enter_context(tc.tile_pool(name="ps_agg", bufs=3, space="PSUM"))
        ps_deg = ctx.enter_context(tc.tile_pool(name="ps_deg", bufs=3, space="PSUM"))
        ps_out = ctx.enter_context(tc.tile_pool(name="ps_out", bufs=2, space="PSUM"))

        x_all = const_pool.tile([P, n_jt, d], f32r)
        nc.scalar.dma_start(x_all[:], x_d[:])
        ones = const_pool.tile([P, 1], f32r)
        nc.scalar.dma_start(ones[:], ones_d[:])
        u_sb = const_pool.tile([P, d], f32)
        nc.scalar.dma_start(u_sb[:], u_d[:])

        def emit_tail(q, agg_ps, deg_ps):
            """Round tail: 1/deg scale of aggT, U-matmuls, ReLU, store.
            Emitted one round late so the PE FIFO never stalls on it."""
            recip = small_pool.tile([1, w], f32, tag="recip")
            nc.vector.reciprocal_approx_fast(recip[:], deg_ps[:])
            rb = scale_pool.tile([P, w], f32, tag="rb")
            nc.gpsimd.partition_broadcast(rb[:], recip[:])
            aggs = scale_pool.tile([P, w], f32, tag="aggs")
            nc.vector.tensor_mul(aggs[:], agg_ps[:], rb[:])
            out_sb = out_pool.tile([P, n_ot, d], f32, tag="osb")
            for t in range(n_ot):
                o_ps = ps_out.tile([P, d], f32, tag="o2")
                nc.tensor.matmul(
                    o_ps[:],
                    aggs[:, t * d : (t + 1) * d],
                    u_sb[:],
                    start=True,
                    stop=True,
                )
                nc.vector.tensor_relu(out_sb[:, t, :], o_ps[:])
            nc.scalar.dma_start(out_d[:, q * n_ot : (q + 1) * n_ot, :], out_sb[:])

        pending = None
        for q in range(n_rounds):
            agg_ps = ps_agg.tile([P, w], f32, tag="agg")
            deg_ps = ps_deg.tile([1, w], f32, tag="deg")
            # Last round streams in half-size chunks so the compute trailing
            # the final DMA (its chunk's matmuls + the scale/store chain) is
            # as short as possible.
            if q == n_rounds - 1 and jt_per_dma % 2 == 0:
                chunk_jts = [jt_per_dma // 2] * (2 * n_dma)
            else:
                chunk_jts = [jt_per_dma] * n_dma
            jt0 = 0
            for c, cjt in enumerate(chunk_jts):
                adj_sb = adj_pool.tile([P, cjt, w], f32r, tag="adj")
                src = adj_d[
                    jt0 * P : (jt0 + cjt) * P,
                    q * w : (q + 1) * w,
                ].rearrange("(t p) i -> p t i", p=P)
                nc.sync.dma_start(adj_sb[:], src)
                first, last = c == 0, c == len(chunk_jts) - 1
                for u in range(cjt):
                    nc.tensor.matmul(
                        deg_ps[:],
                        ones[:],
                        adj_sb[:, u, :],
                        start=(first and u == 0),
                        stop=(last and u == cjt - 1),
                    )
                for u in range(cjt):
                    nc.tensor.matmul(
                        agg_ps[:],
                        x_all[:, jt0 + u, :],
                        adj_sb[:, u, :],
                        start=(first and u == 0),
                        stop=(last and u == cjt - 1),
                    )
                jt0 += cjt
            if pending is not None:
                emit_tail(*pending)
            pending = (q, agg_ps, deg_ps)
        emit_tail(*pending)

    nc.compile()
    return nc


def _get_program():
    global _PROG
    if _PROG is None:
        _PROG = _build_program()
    return _PROG


def _shard_inputs(x, adj_mat, U):
    i_core = N // 2
    ones_c = np.ones((P, 1), dtype=np.float32)
    in_maps = []
    for c in range(N_CORES):
        b, half = c // 2, c % 2
        i0 = half * i_core
        in_maps.append(
            {
                "adj_s": np.ascontiguousarray(adj_mat[b, :, i0 : i0 + i_core]),
                "x_sp": np.ascontiguousarray(
                    x[b].reshape(N // P, P, D).transpose(1, 0, 2)
                ),
                "U": np.ascontiguousarray(U),
                "ones_c": ones_c,
            }
        )
    return in_maps


def _run(x, adj_mat, U, trace=False):
    from concourse.bass_utils import run_bass_kernel_spmd

    nc = _get_program()
    in_maps = _shard_inputs(x, adj_mat, U)
    res = run_bass_kernel_spmd(
        nc, in_maps, core_ids=list(range(N_CORES)), trace=trace
    )
    i_core = N // 2
    out = np.empty((B, N, D), dtype=np.float32)
    for c in range(N_CORES):
        b, half = c // 2, c % 2
        i0 = half * i_core
        osp = res.results[c]["out_sp"]
        out[b, i0 : i0 + i_core, :] = osp.transpose(1, 0, 2).reshape(i_core, D)
    return out, res


def kernel(x, adj_mat, U):
    out, _ = _run(
        np.asarray(x, dtype=np.float32),
        np.asarray(adj_mat, dtype=np.float32),
        np.asarray(U, dtype=np.float32),
    )
    return out



# revision 2
# speedup vs baseline: 1.0618x; 1.0618x over previous
"""GNN message-passing ConvNet layer on 8 TRN2 NeuronCores (Bass/Tile).

Computes, for x [B=4, N=4096, D=128], adj_mat [B, N, N] (0/1 floats),
U [D, D]:
    deg[b, i] = sum_j adj_mat[b, j, i]
    agg[b, i, :] = sum_j adj[b, j, i] * x[b, j, :]
    out = relu((agg @ U) / deg[..., None])

Sharding: core c handles batch c//2 and destination-node half c%2 — no
collectives; each core reads its own adjacency column slice once.

V1 kernel (per core, memory-bound):
  - Associativity: (A^T x) U == A^T (x U). Precompute y = x @ U once on
    the PE (f32r, ~3.5 us), quantize to bf16. The adjacency pass then
    produces the final pre-relu output directly - no U-matmul tail.
  - adj is 0/1 so it is cast to bf16 on the host (exact) and host-packed
    into the exact SBUF tile stream order [q, p, jt, n] so every DMA is
    a contiguous 8 KiB/partition line. Halves HBM traffic vs fp32.
  - Per i-round of 512: 32 agg matmuls (y_jt stationary, adj moving) into
    PSUM [e,i] + 32 deg matmuls (ones stationary). Tail: fast reciprocal
    of deg, gpsimd partition-broadcast, ScalarE relu, DVE scale, store.
"""

import os
import sys

for _p in ("/opt/trn_rl_repo",):
    if _p not in sys.path and os.path.isdir(_p):
        sys.path.insert(0, _p)

from contextlib import ExitStack

import numpy as np

B, N, D = 4, 4096, 128
P = 128
N_CORES = 8

_PROG = None


def _build_program(n=N, i_core=N // 2, d=D, w=512, jt_per_dma=8):
    from concourse import mybir, tile, bacc

    f32 = mybir.dt.float32
    f32r = mybir.dt.float32r
    bf16 = mybir.dt.bfloat16
    n_jt = n // P              # 32 j-tiles of 128
    n_rounds = i_core // w     # 4 i-rounds of 512
    n_chunks = n_jt // jt_per_dma

    nc = bacc.Bacc(
        "TRN2",
        target_bir_lowering=False,
        debug=False,
        enable_asserts=True,
        num_devices=N_CORES,
    )
    # host-packed: adj_p[q, p, u, n] = adj[b, u*128+p, i0 + q*512 + n]
    adj_d = nc.dram_tensor("adj_p", [n_rounds, P, n_jt, w], bf16, kind="ExternalInput")
    # host-packed transpose: xT_p[d, t, j] = x[b, t*128+j, d]
    xt_d = nc.dram_tensor("xT_p", [P, n_jt, d], f32r, kind="ExternalInput")
    u_d = nc.dram_tensor("U", [d, d], f32r, kind="ExternalInput")
    ones_d = nc.dram_tensor("ones_c", [P, 1], bf16, kind="ExternalInput")
    # out_sp[q, e, n] = out[b, i0 + q*512 + n, e]  (host transposes back)
    out_d = nc.dram_tensor("out_sp", [n_rounds, d, w], f32, kind="ExternalOutput")

    with tile.TileContext(nc, trace_sim=False) as tc, ExitStack() as ctx:
        const_pool = ctx.enter_context(tc.tile_pool(name="const", bufs=1))
        y_pool = ctx.enter_context(tc.tile_pool(name="y", bufs=1))
        adj_pool = ctx.enter_context(tc.tile_pool(name="adj", bufs=3))
        scale_pool = ctx.enter_context(tc.tile_pool(name="scale", bufs=2))
        out_pool = ctx.enter_context(tc.tile_pool(name="out", bufs=4))
        small_pool = ctx.enter_context(tc.tile_pool(name="small", bufs=2))
        ps_y = ctx.enter_context(tc.tile_pool(name="ps_y", bufs=2, space="PSUM"))
        ps_out = ctx.enter_context(tc.tile_pool(name="ps_out", bufs=2, space="PSUM"))
        ps_deg = ctx.enter_context(tc.tile_pool(name="ps_deg", bufs=2, space="PSUM"))

        xt_sb = const_pool.tile([P, n_jt, d], f32r)
        nc.scalar.dma_start(xt_sb[:], xt_d[:])
        u_sb = const_pool.tile([P, d], f32r)
        nc.scalar.dma_start(u_sb[:], u_d[:])
        ones = const_pool.tile([P, 1], bf16)
        nc.scalar.dma_start(ones[:], ones_d[:])

        # Phase 0: y = x @ U, quantized to bf16, laid out [j_in_tile, t, e].
        y_sb = y_pool.tile([P, n_jt, d], bf16)
        for t in range(n_jt):
            y_ps = ps_y.tile([P, d], f32, tag="y")
            nc.tensor.matmul(y_ps[:], xt_sb[:, t, :], u_sb[:], start=True, stop=True)
            nc.vector.tensor_copy(y_sb[:, t, :], y_ps[:])

        # Phase 1: stream adjacency once; agg and deg matmuls per tile.
        for q in range(n_rounds):
            out_ps = ps_out.tile([P, w], f32, tag="o")
            deg_ps = ps_deg.tile([1, w], f32, tag="deg")
            for c in range(n_chunks):
                adj_sb = adj_pool.tile([P, jt_per_dma, w], bf16, tag="adj")
                nc.sync.dma_start(
                    adj_sb[:],
                    adj_d[q, :, c * jt_per_dma : (c + 1) * jt_per_dma, :],
                )
                first, last = c == 0, c == n_chunks - 1
                for u in range(jt_per_dma):
                    t = c * jt_per_dma + u
                    nc.tensor.matmul(
                        out_ps[:],
                        y_sb[:, t, :],
                        adj_sb[:, u, :],
                        start=(first and u == 0),
                        stop=(last and u == jt_per_dma - 1),
                    )
                for u in range(jt_per_dma):
                    nc.tensor.matmul(
                        deg_ps[:],
                        ones[:],
                        adj_sb[:, u, :],
                        start=(first and u == 0),
                        stop=(last and u == jt_per_dma - 1),
                    )
            # Tail: out = relu(out_ps) * (1/deg) broadcast over partitions.
            recip = small_pool.tile([1, w], f32, tag="recip")
            nc.vector.reciprocal_approx_fast(recip[:], deg_ps[:])
            rb = scale_pool.tile([P, w], f32, tag="rb")
            nc.gpsimd.partition_broadcast(rb[:], recip[:])
            relu_sb = out_pool.tile([P, w], f32, tag="relu")
            nc.scalar.activation(
                relu_sb[:], out_ps[:], mybir.ActivationFunctionType.Relu
            )
            out_sb = out_pool.tile([P, w], f32, tag="osb")
            nc.vector.tensor_mul(out_sb[:], relu_sb[:], rb[:])
            nc.scalar.dma_start(out_d[q, :, :], out_sb[:])

    nc.compile()
    return nc


def _get_program():
    global _PROG
    if _PROG is None:
        _PROG = _build_program()
    return _PROG


def _shard_inputs(x, adj_mat, U):
    import ml_dtypes

    bf16 = ml_dtypes.bfloat16
    i_core = N // 2
    n_jt = N // P
    n_rounds = i_core // 512
    ones_c = np.ones((P, 1), dtype=bf16)
    u_f = np.ascontiguousarray(U, dtype=np.float32)
    adj_bf = adj_mat.astype(bf16)  # exact: values are 0/1
    in_maps = []
    for c in range(N_CORES):
        b, half = c // 2, c % 2
        i0 = half * i_core
        # [N, i_core] -> [u, p, q, n] -> [q, p, u, n]
        a = adj_bf[b, :, i0 : i0 + i_core].reshape(n_jt, P, n_rounds, 512)
        a = np.ascontiguousarray(a.transpose(2, 1, 0, 3))
        xt = np.ascontiguousarray(
            x[b].reshape(n_jt, P, D).transpose(2, 0, 1), dtype=np.float32
        )
        in_maps.append(
            {"adj_p": a, "xT_p": xt, "U": u_f, "ones_c": ones_c}
        )
    return in_maps


def _run(x, adj_mat, U, trace=False):
    from concourse.bass_utils import run_bass_kernel_spmd

    nc = _get_program()
    in_maps = _shard_inputs(x, adj_mat, U)
    res = run_bass_kernel_spmd(
        nc, in_maps, core_ids=list(range(N_CORES)), trace=trace
    )
    i_core = N // 2
    out = np.empty((B, N, D), dtype=np.float32)
    for c in range(N_CORES):
        b, half = c // 2, c % 2
        i0 = half * i_core
        osp = res.results[c]["out_sp"]  # [q, e, n]
        out[b, i0 : i0 + i_core, :] = osp.transpose(0, 2, 1).reshape(i_core, D)
    return out, res


def kernel(x, adj_mat, U):
    out, _ = _run(
        np.asarray(x, dtype=np.float32),
        np.asarray(adj_mat, dtype=np.float32),
        np.asarray(U, dtype=np.float32),
    )
    return out
